# revision 18
# baseline (speedup 1.0000x reference)
"""Trainium2 Bass kernel for nn_CombinedLoss (pose + point-cloud + flow loss).

Self-contained: accepts FULL inputs, shards across 8 NeuronCores internally,
returns the FULL output (5-tuple of f32 scalars, matching the reference).

Sharding strategy (v3):
  - flow tensors [B,1000,2,32,64]: sharded along the 1000-iteration axis
    (125 iters/core), partition dim = t, so the gamma-weight is a
    per-partition scalar.  pred/gt stored fp8-e4m3, valid bf16.
  - DMA: the flow data is transferred as 8 large CONTIGUOUS per-b blocks
    (pg_b [125,8192] fp8, v_b [125,2048] bf16) issued alternately from the
    scalar and gpsimd queues (each spreads across all 16 DMA engines at
    ~360GB/s aggregate; the sync queue only reaches 5 engines, so it only
    carries tiny transfers).
  - flow compute per b: DVE does the fp8 subtract (1x mode) and most of the
    bf16 mask-multiplies (2x mode); a few muls go to GpSimd; ScalarE does
    |d|*w with fused row-accumulate.
  - activation tables: Arctan (pose) runs first on the trig table, then one
    switch to sqrt_and_others covers Abs+Square+Sqrt+Identity for the whole
    flow + point-cloud phase; sqrt(rat2) for the pose uses GpSimd pow(0.5)
    so no third table load is needed.
  - point_clouds [B,4,N]: sharded along N (12500 pts/core), batch-stacked
    into [128, 1568] so one matmul applies all four (M_b - I) transforms.
  - pose math: host packs sign-permuted copies of target_rot so each
    quaternion product is 4 tensor_scalar ops + 1 strided reduce; runs on
    Pool so DVE stays free for flow.
Each core emits 5 partial scalars; the host sums partials across cores
(the all-reduce) and takes core 0's value for the replicated pose terms.
"""

import os

import numpy as np

import concourse.bass as bass
import concourse.bacc as bacc
import concourse.mybir as mybir
import concourse.tile as tile

N_CORES = 8
B = 4
N_PTS = 100000
N_ITERS = 1000
H, W = 32, 64
GAMMA = 0.8

T_PER_CORE = N_ITERS // N_CORES          # 125 flow iters per core
T_PAD = 128                              # padded to 128 rows: a DMA uses
                                         # largest-divisor(P)<=16 engines, so
                                         # 128 rows -> all 16 (125 -> only 5)
ROWS = B * T_PAD                         # 512 = b-major rows of [128 t]
FLOW_MEAN_DEN = B * 2 * H * W            # 16384 (mean denominator per iter)
N_Q = 8                                  # 4 b-chunks x 2 c-halves
PTS_PER_CORE = N_PTS // N_CORES          # 12500
PC_GROUPS = 8                            # point groups -> 128 matmul rows
PC_COLS = 1568                           # padded 12544 / 8 groups
PAD_N = PC_GROUPS * PC_COLS              # 12544 (pads with zero points)

F32 = mybir.dt.float32
BF16 = mybir.dt.bfloat16
FP8 = mybir.dt.float8e4
AF = mybir.ActivationFunctionType
OP = mybir.AluOpType
AX = mybir.AxisListType

HALF_PI = float(np.pi / 2.0)

# per-chunk engine for the mask multiply ("v"=DVE, "g"=GpSimd); chunk
# index iq = 2*b + h.  GpSimd muls relieve the DVE (the bottleneck) at
# ~4.2us/chunk of otherwise-idle Pool time.
MUL_ENG = ["v", "v", "v", "g", "v", "v", "g", "v"]


def build_nc():
    nc = bacc.Bacc("TRN2", target_bir_lowering=False, debug=False,
                   num_devices=N_CORES)

    # pg row r = b*128 + t (3 zero pad rows per b); cols [pred-c0 | gt-c0 | pred-c1 | gt-c1] x 2048
    pg = nc.dram_tensor("pg", [ROWS, 8192], FP8, kind="ExternalInput")
    valid = nc.dram_tensor("valid", [ROWS, 2048], BF16, kind="ExternalInput")
    wrow = nc.dram_tensor("wrow", [T_PAD, 1], F32, kind="ExternalInput")
    pc = nc.dram_tensor("pc", [16 * PC_GROUPS, PC_COLS], F32, kind="ExternalInput")
    smalls = nc.dram_tensor("smalls", [B, 70], F32, kind="ExternalInput")
    out = nc.dram_tensor("out", [1, 5], F32, kind="ExternalOutput")

    with tile.TileContext(nc) as tc:
        _body(nc, tc, pg, valid, wrow, pc, smalls, out)
    nc.compile()
    return nc


def _body(nc, tc, pg, valid, wrow, pc, smalls, out):
    with (
        tc.tile_pool(name="small", bufs=1) as small,
        tc.tile_pool(name="vpool", bufs=4) as vpool,
        tc.tile_pool(name="pgpool", bufs=4) as pgpool,
        tc.tile_pool(name="dpool", bufs=4) as dpool,
        tc.tile_pool(name="pcpool", bufs=1) as pcpool,
        tc.tile_pool(name="pwork", bufs=3) as pwork,
        tc.tile_pool(name="psum_d", bufs=2, space="PSUM") as psum_d,
        tc.tile_pool(name="psum_e", bufs=2, space="PSUM") as psum_e,
        tc.tile_pool(name="psum_s", bufs=1, space="PSUM") as psum_s,
        tc.tile_pool(name="dram", bufs=1, space="DRAM") as dram,
    ):
        cnt = [0]

        def st(p_, f_, tag=None, dt=F32):
            cnt[0] += 1
            nm = tag or f"s{cnt[0]}"
            return small.tile([p_, f_], dt, name=nm, tag=nm)

        # ---------------- tiny input DMAs on the sync queue ----------------
        sm = st(B, 70, tag="sm")
        nc.sync.dma_start(sm[:], smalls[:])
        wr = st(T_PAD, 1, tag="wr")
        nc.sync.dma_start(wr[:], wrow[:])

        import ml_dtypes
        l3_np = np.zeros((128, 32), dtype=ml_dtypes.bfloat16)
        for g in range(PC_GROUPS):
            for b in range(B):
                for i in range(4):
                    l3_np[16 * g + 4 * b + i, 4 * g + b] = 1.0
        l3_dram = nc.inline_tensor(np.asarray(l3_np), name="l3_const")
        lhsT3 = st(128, 32, tag="lhsT3", dt=BF16)
        nc.sync.dma_start(lhsT3[:], l3_dram[:])

        # ---------------- bulk flow DMAs: scalar + gpsimd queues -----------
        # pg rows b*128..(b+1)*128 are a fully-contiguous [128, 8192] block;
        # 128 partition-rows let the transfer stripe over all 16 DMA engines
        # (engine count = largest divisor of P <= 16; 125 would use only 5).
        pg_ts = [pgpool.tile([T_PAD, 8192], FP8, tag="pgb",
                             name=f"pg{b}") for b in range(B)]
        v_ts = [vpool.tile([T_PAD, 2048], BF16, tag="v", name=f"v{b}")
                for b in range(B)]

        def pg_dma(eng, b, h):
            # per-channel-half [128, 4096] so compute starts on the first half
            r0 = b * T_PAD
            eng.dma_start(pg_ts[b][:, h * 4096:(h + 1) * 4096],
                          pg[r0:r0 + T_PAD, h * 4096:(h + 1) * 4096])

        def v_dma(eng, b):
            r0 = b * T_PAD
            eng.dma_start(v_ts[b][:], valid[r0:r0 + T_PAD, :])

        pg_dma(nc.gpsimd, 0, 0)
        v_dma(nc.scalar, 0)
        pg_dma(nc.gpsimd, 0, 1)
        pg_dma(nc.scalar, 1, 0)
        pg_dma(nc.scalar, 1, 1)
        v_dma(nc.gpsimd, 1)
        pcp = pcpool.tile([128, PC_COLS], F32, tag="pcp")
        nc.scalar.dma_start(pcp[:], pc[:])
        pg_dma(nc.gpsimd, 2, 0)
        pg_dma(nc.gpsimd, 2, 1)
        v_dma(nc.scalar, 2)
        pg_dma(nc.scalar, 3, 0)
        pg_dma(nc.scalar, 3, 1)
        v_dma(nc.gpsimd, 3)

        tt_s, tr_s = sm[:, 0:3], sm[:, 3:7]
        te_s, re_s = sm[:, 7:10], sm[:, 10:14]
        ablk, bblk = sm[:, 14:30], sm[:, 30:46]
        rep16, retr8 = sm[:, 46:62], sm[:, 62:70]

        # ============ DVE-early: norms/recips + u (before flow data) =======
        sef = st(B, 8)
        nc.vector.tensor_mul(sef[:], retr8, retr8)
        nn = st(B, 2)   # [n2f, n2e]
        nc.vector.tensor_reduce(nn[:], sef[:].rearrange("b (g k) -> b g k", k=4),
                                AX.X, OP.add)
        rr = st(B, 2)   # [1/n2f, 1/n2e]
        nc.vector.reciprocal(rr[:], nn[:])
        rnq2 = st(B, 1)  # 2/(n2e*n2f)
        nc.vector.tensor_mul(rnq2[:], rr[:, 0:1], rr[:, 1:2])
        nc.vector.tensor_scalar(rnq2[:], rnq2[:], 2.0, None, OP.mult)
        rne2 = st(B, 1)  # 2/n2e
        nc.vector.tensor_scalar(rne2[:], rr[:, 1:2], 2.0, None, OP.mult)
        u = st(B, 3)
        nc.vector.tensor_sub(u[:], tt_s, te_s)
        z128 = st(128, 128, tag="z128")
        nc.vector.memset(z128[:], 0.0)
        l2d = dram.tile([128, 128], F32)
        nc.sync.dma_start(l2d[:], z128[:])

        # ============ Pool: quat products + A(qm) ==========================
        # off-diagonal col targets / G sources (col layout 4j+i, baseline)
        def build_A_offdiag(eng, E_t, G):
            eng.tensor_sub(E_t[:, 4:5], G[1][:, 2:3], G[0][:, 3:4])   # A01
            eng.tensor_add(E_t[:, 8:9], G[1][:, 3:4], G[0][:, 2:3])   # A02
            eng.tensor_add(E_t[:, 1:2], G[1][:, 2:3], G[0][:, 3:4])   # A10
            eng.tensor_sub(E_t[:, 9:10], G[2][:, 3:4], G[0][:, 1:2])  # A12
            eng.tensor_sub(E_t[:, 2:3], G[1][:, 3:4], G[0][:, 2:3])   # A20
            eng.tensor_add(E_t[:, 6:7], G[2][:, 3:4], G[0][:, 1:2])   # A21

        def build_A_diag(eng, E_t, G):
            # Aii = G0[0] + Gi[i] - 2  for i=1..3 at cols {0,5,10}
            for col, Gi, gi in ((0, G[1], 1), (5, G[2], 2), (10, G[3], 3)):
                tdg = st(B, 1)
                eng.tensor_add(tdg[:], G[0][:, 0:1], Gi[:, gi:gi + 1])
                eng.tensor_scalar(E_t[:, col:col + 1], tdg[:], -2.0,
                                  None, OP.add)

        def build_G(eng, q_t, inv2_t):
            # Gk = q * (q_k * 2/n2) via dual-scalar tensor_scalar
            G = []
            for k in range(4):
                Gk = st(B, 4)
                eng.tensor_scalar(Gk[:], q_t[:], q_t[:, k:k + 1], inv2_t,
                                  OP.mult, OP.mult)
                G.append(Gk)
            return G

        # A(re) + translation column first (feeds lhsT2 via Mt); the
        # rot-angle chain after (only needed at the tail)
        AE = st(B, 16, tag="AE")
        nc.gpsimd.memset(AE[:], 0.0)
        Gr = build_G(nc.gpsimd, re_s, rne2[:])
        build_A_offdiag(nc.gpsimd, AE[:], Gr)
        build_A_diag(nc.gpsimd, AE[:], Gr)
        E = st(B, 16, tag="E")
        nc.gpsimd.memset(E[:], 0.0)
        # Mt = u + Ae^T u expanded with per-partition scalars (no STT on Pool)
        aev = AE[:].rearrange("b (j i) -> b j i", i=4)
        mt1 = st(B, 3)
        nc.gpsimd.tensor_scalar(mt1[:], aev[:, 0:3, 0], u[:, 0:1], None,
                                OP.mult)
        mt2 = st(B, 3)
        nc.gpsimd.tensor_scalar(mt2[:], aev[:, 0:3, 1], u[:, 1:2], None,
                                OP.mult)
        nc.gpsimd.tensor_add(mt1[:], mt1[:], mt2[:])
        nc.gpsimd.tensor_scalar(mt2[:], aev[:, 0:3, 2], u[:, 2:3], None,
                                OP.mult)
        nc.gpsimd.tensor_add(mt1[:], mt1[:], mt2[:])
        nc.gpsimd.tensor_add(E[:, 12:15], mt1[:], u[:])

        SP = st(B, 16, tag="SPr")
        nc.gpsimd.tensor_mul(SP[:], ablk, rep16)
        twxyz = st(B, 4, tag="twxyz")
        nc.gpsimd.tensor_add(twxyz[:], SP[:, 0:4], SP[:, 4:8])
        nc.gpsimd.tensor_add(twxyz[:], twxyz[:], SP[:, 8:12])
        nc.gpsimd.tensor_add(twxyz[:], twxyz[:], SP[:, 12:16])
        SQ = st(B, 16, tag="SPq")
        nc.gpsimd.tensor_mul(SQ[:], bblk, rep16)
        qm = st(B, 4, tag="qm")
        nc.gpsimd.tensor_add(qm[:], SQ[:, 0:4], SQ[:, 4:8])
        nc.gpsimd.tensor_add(qm[:], qm[:], SQ[:, 8:12])
        nc.gpsimd.tensor_add(qm[:], qm[:], SQ[:, 12:16])
        Gq = build_G(nc.gpsimd, qm[:], rnq2[:])
        build_A_offdiag(nc.gpsimd, E[:], Gq)
        build_A_diag(nc.gpsimd, E[:], Gq)
        sqt = st(B, 4)
        nc.gpsimd.tensor_mul(sqt[:], twxyz[:], twxyz[:])
        vn2 = st(B, 1)
        nc.gpsimd.tensor_add(vn2[:], sqt[:, 1:2], sqt[:, 2:3])
        nc.gpsimd.tensor_add(vn2[:], vn2[:], sqt[:, 3:4])
        aw2 = sqt[:, 0:1]

        # lhsT2 bounce right after E completes: replicate E to 8 group
        # copies (3 doubling copies on Pool), then ONE 4-level scatter +
        # reload -- vs 8 serial scatters whose sem-props cost ~12us.
        E8 = st(B, 128, tag="E8")
        nc.gpsimd.tensor_scalar(E8[:, 0:16], E[:], 1.0, None, OP.mult)
        nc.gpsimd.tensor_scalar(E8[:, 16:32], E8[:, 0:16], 1.0, None, OP.mult)
        nc.gpsimd.tensor_scalar(E8[:, 32:64], E8[:, 0:32], 1.0, None, OP.mult)
        nc.gpsimd.tensor_scalar(E8[:, 64:128], E8[:, 0:64], 1.0, None, OP.mult)
        e8v = E8[:].rearrange("b (g j i) -> b g j i", j=4, i=4)
        l2d_ap = l2d[:]
        dstg = bass.AP(l2d_ap.tensor, 0,
                       [[2064, PC_GROUPS], [516, 4], [128, 4], [1, 4]])
        nc.sync.dma_start(dstg, e8v)
        lhsT2 = st(128, 128, tag="lhsT2")
        nc.sync.dma_start(lhsT2[:], l2d[:])

        # ================== flow + wedged pose work on DVE =================
        rsall = st(T_PAD, N_Q, tag="rsall")
        acc32 = st(32, 1, tag="acc32")
        nc.vector.memset(acc32[:], 0.0)
        dsq = pcpool.tile([128, PC_COLS], BF16, tag="dsq")
        col_chunks = [(0, 512), (512, 1024), (1024, 1536), (1536, PC_COLS)]

        def flow_b(b):
            pgb = pg_ts[b]
            d_t = dpool.tile([T_PAD, 4096], BF16, tag="d")
            for h in range(2):
                c0 = h * 2048
                nc.vector.tensor_sub(d_t[:, c0:c0 + 2048],
                                     pgb[:, 2 * c0:2 * c0 + 2048],
                                     pgb[:, 2 * c0 + 2048:2 * c0 + 4096])
            for h in range(2):
                c0 = h * 2048
                nc.vector.tensor_mul(d_t[:, c0:c0 + 2048],
                                     d_t[:, c0:c0 + 2048], v_ts[b][:])
            for h in range(2):
                iq = 2 * b + h
                c0 = h * 2048
                scr = dpool.tile([T_PAD, 2048], BF16, tag="scr")
                nc.scalar.activation(scr[:], d_t[:, c0:c0 + 2048], AF.Abs,
                                     scale=wr[:],
                                     accum_out=rsall[:, iq:iq + 1])

        flow_b(0)
        flow_b(1)
        flow_b(2)
        flow_b(3)

        # ---- DVE-late: loss_transl + rot-angle prep ----------------------
        dlt = st(B, 3)
        nc.vector.tensor_sub(dlt[:], te_s, tt_s)
        sc1 = st(B, 3)
        nc.vector.tensor_mul(sc1[:], dlt[:], dlt[:])
        s12 = st(B, 2)
        nc.vector.tensor_reduce(s12[:, 0:1], sc1[:], AX.X, OP.add)
        adl = st(B, 3)
        nc.vector.scalar_tensor_tensor(adl[:], dlt[:], -1.0, dlt[:],
                                       OP.mult, OP.max)
        rdl = st(B, 3)
        nc.vector.tensor_scalar(rdl[:], adl[:], 1.0, 0.0, OP.subtract, OP.max)
        sc2 = st(B, 3)
        nc.vector.tensor_mul(sc2[:], rdl[:], rdl[:])
        nc.vector.tensor_reduce(s12[:, 1:2], sc2[:], AX.X, OP.add)
        ltd = st(B, 1, tag="ltd")   # 2*smooth_l1 row sum per batch
        nc.vector.tensor_sub(ltd[:], s12[:, 0:1], s12[:, 1:2])

        mn2 = st(B, 1)
        nc.vector.tensor_tensor(mn2[:], vn2[:], aw2, OP.min)
        mx2 = st(B, 1)
        nc.vector.tensor_max(mx2[:], vn2[:], aw2)
        rmx = st(B, 1)
        nc.vector.reciprocal(rmx[:], mx2[:])
        rat2 = st(B, 1)
        nc.vector.tensor_mul(rat2[:], mn2[:], rmx[:])
        mflip = st(B, 1, dt=mybir.dt.int32)
        nc.vector.tensor_tensor(mflip[:], vn2[:], aw2, OP.is_gt)

        # ============ point-cloud chain (PE + ScalarE, post-flow) ==========
        for c0, c1 in col_chunks:
            dps = psum_d.tile([128, 512], F32, tag="dps")
            nc.tensor.matmul(dps[:, :c1 - c0], lhsT2[:], pcp[:, c0:c1],
                             start=True, stop=True)
            nc.scalar.activation(dsq[:, c0:c1], dps[:, :c1 - c0], AF.Square)
        for c0, c1 in col_chunks:
            e2 = psum_e.tile([32, 512], F32, tag="e2")
            nc.tensor.matmul(e2[:, :c1 - c0], lhsT3[:], dsq[:, c0:c1],
                             start=True, stop=True)
            errt = pwork.tile([32, 512], F32, tag="errt")
            ers = pwork.tile([32, 1], F32, tag="ers")
            nc.scalar.activation(errt[:, :c1 - c0], e2[:, :c1 - c0], AF.Sqrt,
                                 accum_out=ers[:])
            nc.gpsimd.tensor_add(acc32[:], acc32[:], ers[:])

        # ---- deferred pose-angle tail (Sqrt in-set, one Arctan load) ------
        rat = st(B, 1)
        nc.scalar.activation(rat[:], rat2[:], AF.Sqrt)
        ang = st(B, 1)
        nc.scalar.activation(ang[:], rat[:], AF.Arctan)
        alt = st(B, 1)
        nc.gpsimd.tensor_scalar(alt[:], ang[:], -1.0, HALF_PI, OP.mult, OP.add)
        rot = st(B, 1, tag="rot")   # atan2 per batch
        nc.vector.select(rot[:], mflip[:], alt[:], ang[:])

        # ================== final reductions ===============================
        ones128 = st(T_PAD, 1, tag="ones128")
        nc.gpsimd.memset(ones128[:], 1.0)
        ones4 = st(B, 1, tag="ones4")
        nc.gpsimd.memset(ones4[:], 1.0)
        ones32 = st(32, 1, tag="ones32")
        nc.gpsimd.memset(ones32[:], 1.0)

        # flow: [128,8] --rowsum--> [128,1] --matmul--> [1,1]
        rsum = st(T_PAD, 1, tag="rsum")
        nc.vector.tensor_reduce(rsum[:], rsall[:], AX.X, OP.add)

        ps = psum_s.tile([1, 5], F32, tag="ps")
        nc.tensor.matmul(ps[:, 0:1], rsum[:], ones128[:], start=True, stop=True)
        nc.tensor.matmul(ps[:, 1:2], acc32[:], ones32[:], start=True, stop=True)
        nc.tensor.matmul(ps[:, 2:3], ltd[:], ones4[:], start=True, stop=True)
        nc.tensor.matmul(ps[:, 3:4], rot[:], ones4[:], start=True, stop=True)

        out5 = st(1, 5, tag="out5")
        # loss_transl = 0.5*sum/4 ; loss_rot = 2*sum/4 ; pc = sum/(B*N) ; flow
        nc.scalar.mul(out5[:, 1:2], ps[:, 2:3], 0.125)
        nc.scalar.mul(out5[:, 2:3], ps[:, 3:4], 0.5)
        nc.scalar.mul(out5[:, 3:4], ps[:, 1:2], 1.0 / (B * N_PTS))
        nc.scalar.copy(out5[:, 4:5], ps[:, 0:1])
        t1 = st(1, 1)
        t2 = st(1, 1)
        nc.gpsimd.tensor_add(t1[:], out5[:, 1:2], out5[:, 2:3])
        nc.gpsimd.tensor_add(t2[:], out5[:, 3:4], out5[:, 4:5])
        nc.gpsimd.tensor_scalar(t1[:], t1[:], 0.5 / N_CORES, None, OP.mult)
        nc.vector.scalar_tensor_tensor(out5[:, 0:1], t2[:], 0.5, t1[:],
                                       OP.mult, OP.add)
        nc.sync.dma_start(out[:], out5[:])


_CACHE = {}
last_results = None


def _get_nc():
    if "nc" not in _CACHE:
        _CACHE["nc"] = build_nc()
    return _CACHE["nc"]


def _signed_blocks(r):
    """r: [B,4] -> [B,32] = sign-permuted copies for the two quat products.

    A-block (rot_err x conj(target_rot), component-ordered):
      A0=(r0,-r1,-r2,-r3)  A1=(r1,r0,r3,-r2)  A2=(r2,-r3,r0,r1)  A3=(r3,r2,-r1,r0)
    B-block (conj(rot_err) x target_rot):
      B0=(r0,r1,r2,r3)  B1=(r1,-r0,r3,-r2)  B2=(r2,-r3,-r0,r1)  B3=(r3,r2,-r1,-r0)
    """
    r0, r1, r2, r3 = r[:, 0:1], r[:, 1:2], r[:, 2:3], r[:, 3:4]
    a = np.concatenate([r0, -r1, -r2, -r3,
                        r1, r0, r3, -r2,
                        r2, -r3, r0, r1,
                        r3, r2, -r1, r0], axis=1)
    b = np.concatenate([r0, r1, r2, r3,
                        r1, -r0, r3, -r2,
                        r2, -r3, -r0, r1,
                        r3, r2, -r1, -r0], axis=1)
    return np.concatenate([a, b], axis=1)


def make_in_maps(point_clouds, target_transl, target_rot, transl_err, rot_err,
                 calib_flow_pred, calib_flow_gt, flow_valid):
    import ml_dtypes
    point_clouds = np.asarray(point_clouds, np.float32)
    calib_flow_pred = np.asarray(calib_flow_pred, np.float32)
    calib_flow_gt = np.asarray(calib_flow_gt, np.float32)
    flow_valid = np.asarray(flow_valid, np.float32)
    tt = np.ascontiguousarray(np.asarray(target_transl, np.float32))
    tr = np.ascontiguousarray(np.asarray(target_rot, np.float32))
    te = np.ascontiguousarray(np.asarray(transl_err, np.float32))
    re = np.ascontiguousarray(np.asarray(rot_err, np.float32))

    w_full = (GAMMA ** (N_ITERS - 1 - np.arange(N_ITERS, dtype=np.float64)))
    w_full = (w_full / FLOW_MEAN_DEN).astype(np.float32)

    re_rep16 = np.repeat(re, 4, axis=1)          # [B,16]: re[i] at 4i..4i+3
    retr8 = np.concatenate([tr, re], axis=1)     # [B,8]
    smalls = np.concatenate([tt, tr, te, re, _signed_blocks(tr),
                             re_rep16, retr8], axis=1).astype(np.float32)

    # [B,1000,2,32,64] -> per-core rows (b,t), cols pred(c,hw) | gt(c,hw)
    pred8 = calib_flow_pred.reshape(B, N_ITERS, 4096).astype(
        ml_dtypes.float8_e4m3)
    gt8 = calib_flow_gt.reshape(B, N_ITERS, 4096).astype(
        ml_dtypes.float8_e4m3)
    valid16 = flow_valid.reshape(B, N_ITERS, 2048).astype(ml_dtypes.bfloat16)

    in_maps = []
    for c in range(N_CORES):
        t0, t1 = c * T_PER_CORE, (c + 1) * T_PER_CORE
        n0, n1 = c * PTS_PER_CORE, (c + 1) * PTS_PER_CORE
        p4 = pred8[:, t0:t1].reshape(B, T_PER_CORE, 2, 2048)
        g4 = gt8[:, t0:t1].reshape(B, T_PER_CORE, 2, 2048)
        pg_s = np.stack([p4[:, :, 0], g4[:, :, 0],
                         p4[:, :, 1], g4[:, :, 1]], axis=2)
        # pad each b-block from 125 to 128 t-rows; the zero rows (and the
        # zero wrow entries) contribute nothing to any reduction
        pg_pad = np.zeros((B, T_PAD, 8192), pg_s.dtype)
        pg_pad[:, :T_PER_CORE] = pg_s.reshape(B, T_PER_CORE, 8192)
        v_pad = np.zeros((B, T_PAD, 2048), valid16.dtype)
        v_pad[:, :T_PER_CORE] = valid16[:, t0:t1]
        w_pad = np.zeros((T_PAD, 1), np.float32)
        w_pad[:T_PER_CORE, 0] = w_full[t0:t1]
        in_maps.append({
            "pg": np.ascontiguousarray(pg_pad).reshape(ROWS, 8192),
            "valid": np.ascontiguousarray(v_pad).reshape(ROWS, 2048),
            "wrow": w_pad,
            "pc": _pack_pc(point_clouds[:, :, n0:n1]),
            "smalls": smalls,
        })
    return in_maps


def _pack_pc(pc_shard):
    """[B,4,12500] -> [128,1568]: row 16g+4b+j = pc[b,j,1568g:1568(g+1)],
    zero-padded to 12544 points (zero points contribute zero error)."""
    pad = np.zeros((B, 4, PAD_N), np.float32)
    pad[:, :, :PTS_PER_CORE] = pc_shard
    v = pad.reshape(B, 4, PC_GROUPS, PC_COLS)
    return np.ascontiguousarray(
        v.transpose(2, 0, 1, 3).reshape(16 * PC_GROUPS, PC_COLS))


def combine_outputs(core_outs):
    """core_outs: [N_CORES, 5] array of per-core partials."""
    core_outs = np.asarray(core_outs, np.float32)
    total = np.float32(core_outs[:, 0].sum())
    lt = np.float32(core_outs[0, 1])
    lr = np.float32(core_outs[0, 2])
    pcb = np.float32(core_outs[:, 3].sum())
    fl = np.float32(core_outs[:, 4].sum())
    return (total, lt, lr, pcb, fl)


def _install_ntff_hook_shim():
    """bass_utils expects antenv.axon_hooks when trace=True under axon;
    this image's antenv lacks it. Provide it and register the ctypes hook."""
    import sys
    import types
    if "antenv.axon_hooks" in sys.modules:
        return
    mod = types.ModuleType("antenv.axon_hooks")
    state = {"hook": None}
    mod.set_axon_ntff_profile_hook = lambda h: state.__setitem__("hook", h)
    mod.get_axon_ntff_profile_hook = lambda: state["hook"]
    sys.modules["antenv.axon_hooks"] = mod
    try:
        import antenv
        antenv.axon_hooks = mod
    except ImportError:
        pass
    try:
        from trn_agent_boot.trn_boot import _ntff_profile_via_ctypes
        mod.set_axon_ntff_profile_hook(
            _ntff_profile_via_ctypes("/opt/axon/libaxon_pjrt.so"))
    except Exception:
        pass


def kernel(point_clouds, target_transl, target_rot, transl_err, rot_err,
           calib_flow_pred, calib_flow_gt, flow_valid):
    global last_results
    from concourse.bass_utils import run_bass_kernel_spmd

    nc = _get_nc()
    in_maps = make_in_maps(point_clouds, target_transl, target_rot,
                           transl_err, rot_err, calib_flow_pred,
                           calib_flow_gt, flow_valid)
    trace = bool(int(os.environ.get("KERNEL_TRACE", "0")))
    kwargs = {}
    if trace:
        _install_ntff_hook_shim()
        kwargs = {"trace": True, "trace_cores": list(range(N_CORES))}
    res = run_bass_kernel_spmd(nc, in_maps, core_ids=list(range(N_CORES)),
                               **kwargs)
    last_results = res
    core_outs = np.stack([res.results[c]["out"][0] for c in range(N_CORES)])
    return combine_outputs(core_outs)


# revision 22
# speedup vs baseline: 1.1873x; 1.1873x over previous
"""Trainium2 Bass kernel for nn_CombinedLoss (pose + point-cloud + flow loss).

Self-contained: accepts FULL inputs, shards across 8 NeuronCores internally,
returns the FULL output (5-tuple of f32 scalars, matching the reference).

Sharding strategy (v3):
  - flow tensors [B,1000,2,32,64]: sharded along the 1000-iteration axis
    (125 iters/core), partition dim = t, so the gamma-weight is a
    per-partition scalar.  pred/gt stored fp8-e4m3, valid bf16.
  - DMA: the flow data is transferred as 8 large CONTIGUOUS per-b blocks
    (pg_b [125,8192] fp8, v_b [125,2048] bf16) issued alternately from the
    scalar and gpsimd queues (each spreads across all 16 DMA engines at
    ~360GB/s aggregate; the sync queue only reaches 5 engines, so it only
    carries tiny transfers).
  - flow compute per b: DVE does the fp8 subtract (1x mode) and most of the
    bf16 mask-multiplies (2x mode); a few muls go to GpSimd; ScalarE does
    |d|*w with fused row-accumulate.
  - activation tables: Arctan (pose) runs first on the trig table, then one
    switch to sqrt_and_others covers Abs+Square+Sqrt+Identity for the whole
    flow + point-cloud phase; sqrt(rat2) for the pose uses GpSimd pow(0.5)
    so no third table load is needed.
  - point_clouds [B,4,N]: sharded along N (12500 pts/core), batch-stacked
    into [128, 1568] so one matmul applies all four (M_b - I) transforms.
  - pose math: host packs sign-permuted copies of target_rot so each
    quaternion product is 4 tensor_scalar ops + 1 strided reduce; runs on
    Pool so DVE stays free for flow.
Each core emits 5 partial scalars; the host sums partials across cores
(the all-reduce) and takes core 0's value for the replicated pose terms.
"""

import os

import numpy as np

import concourse.bass as bass
import concourse.bacc as bacc
import concourse.mybir as mybir
import concourse.tile as tile

N_CORES = 8
B = 4
N_PTS = 100000
N_ITERS = 1000
H, W = 32, 64
GAMMA = 0.8

T_PER_CORE = N_ITERS // N_CORES          # 125 flow iters per core
T_PAD = 128                              # padded to 128 rows: a DMA uses
                                         # largest-divisor(P)<=16 engines, so
                                         # 128 rows -> all 16 (125 -> only 5)
ROWS = B * T_PAD                         # 512 = b-major rows of [128 t]
FLOW_MEAN_DEN = B * 2 * H * W            # 16384 (mean denominator per iter)
N_Q = 8                                  # 4 b-chunks x 2 c-halves
PTS_PER_CORE = N_PTS // N_CORES          # 12500
PC_GROUPS = 8                            # point groups -> 128 matmul rows
PC_COLS = 1568                           # padded 12544 / 8 groups
PAD_N = PC_GROUPS * PC_COLS              # 12544 (pads with zero points)

F32 = mybir.dt.float32
BF16 = mybir.dt.bfloat16
FP8 = mybir.dt.float8e4
AF = mybir.ActivationFunctionType
OP = mybir.AluOpType
AX = mybir.AxisListType

HALF_PI = float(np.pi / 2.0)

# per-chunk engine for the mask multiply ("v"=DVE, "g"=GpSimd); chunk
# index iq = 2*b + h.  GpSimd muls relieve the DVE (the bottleneck) at
# ~4.2us/chunk of otherwise-idle Pool time.
MUL_ENG = ["v", "v", "v", "g", "v", "v", "g", "v"]


def build_nc():
    nc = bacc.Bacc("TRN2", target_bir_lowering=False, debug=False,
                   num_devices=N_CORES)

    # pg row r = b*128 + t (3 zero pad rows per b); cols [pred-c0 | gt-c0 | pred-c1 | gt-c1] x 2048
    pg = nc.dram_tensor("pg", [ROWS, 8192], FP8, kind="ExternalInput")
    valid = nc.dram_tensor("valid", [ROWS, 2048], BF16, kind="ExternalInput")
    wrow = nc.dram_tensor("wrow", [T_PAD, 1], F32, kind="ExternalInput")
    pc = nc.dram_tensor("pc", [16 * PC_GROUPS, PC_COLS], F32, kind="ExternalInput")
    smalls = nc.dram_tensor("smalls", [B, 70], F32, kind="ExternalInput")
    out = nc.dram_tensor("out", [1, 5], F32, kind="ExternalOutput")

    with tile.TileContext(nc) as tc:
        _body(nc, tc, pg, valid, wrow, pc, smalls, out)
    nc.compile()
    return nc


def _body(nc, tc, pg, valid, wrow, pc, smalls, out):
    with (
        tc.tile_pool(name="small", bufs=1) as small,
        tc.tile_pool(name="vpool", bufs=4) as vpool,
        tc.tile_pool(name="pgpool", bufs=4) as pgpool,
        tc.tile_pool(name="dpool", bufs=4) as dpool,
        tc.tile_pool(name="pcpool", bufs=1) as pcpool,
        tc.tile_pool(name="pwork", bufs=3) as pwork,
        tc.tile_pool(name="psum_d", bufs=2, space="PSUM") as psum_d,
        tc.tile_pool(name="psum_e", bufs=2, space="PSUM") as psum_e,
        tc.tile_pool(name="psum_s", bufs=1, space="PSUM") as psum_s,
        tc.tile_pool(name="dram", bufs=1, space="DRAM") as dram,
    ):
        cnt = [0]

        def st(p_, f_, tag=None, dt=F32):
            cnt[0] += 1
            nm = tag or f"s{cnt[0]}"
            return small.tile([p_, f_], dt, name=nm, tag=nm)

        # ---------------- tiny input DMAs on the sync queue ----------------
        sm = st(B, 70, tag="sm")
        nc.sync.dma_start(sm[:], smalls[:])
        wr = st(T_PAD, 1, tag="wr")
        nc.sync.dma_start(wr[:], wrow[:])

        import ml_dtypes
        l3_np = np.zeros((128, 32), dtype=ml_dtypes.bfloat16)
        for g in range(PC_GROUPS):
            for b in range(B):
                for i in range(4):
                    l3_np[16 * g + 4 * b + i, 4 * g + b] = 1.0
        l3_dram = nc.inline_tensor(np.asarray(l3_np), name="l3_const")
        lhsT3 = st(128, 32, tag="lhsT3", dt=BF16)
        nc.sync.dma_start(lhsT3[:], l3_dram[:])

        # ---------------- bulk flow DMAs: scalar + gpsimd queues -----------
        # pg rows b*128..(b+1)*128 are a fully-contiguous [128, 8192] block;
        # 128 partition-rows let the transfer stripe over all 16 DMA engines
        # (engine count = largest divisor of P <= 16; 125 would use only 5).
        pg_ts = [pgpool.tile([T_PAD, 8192], FP8, tag="pgb",
                             name=f"pg{b}") for b in range(B)]
        v_ts = [vpool.tile([T_PAD, 2048], BF16, tag="v", name=f"v{b}")
                for b in range(B)]

        def pg_dma(eng, b):
            r0 = b * T_PAD
            eng.dma_start(pg_ts[b][:], pg[r0:r0 + T_PAD, :])

        def v_dma(eng, b):
            r0 = b * T_PAD
            eng.dma_start(v_ts[b][:], valid[r0:r0 + T_PAD, :])

        pg_dma(nc.gpsimd, 0)
        v_dma(nc.scalar, 0)
        pg_dma(nc.scalar, 1)
        v_dma(nc.gpsimd, 1)
        pg_dma(nc.gpsimd, 2)
        v_dma(nc.scalar, 2)
        pg_dma(nc.scalar, 3)
        v_dma(nc.gpsimd, 3)
        pcp = pcpool.tile([128, PC_COLS], F32, tag="pcp")
        nc.scalar.dma_start(pcp[:], pc[:])

        tt_s, tr_s = sm[:, 0:3], sm[:, 3:7]
        te_s, re_s = sm[:, 7:10], sm[:, 10:14]
        ablk, bblk = sm[:, 14:30], sm[:, 30:46]
        rep16, retr8 = sm[:, 46:62], sm[:, 62:70]

        # (norms/recips/u deferred into DVE gaps after the first subs so
        # the flow starts the moment pg0 lands)

        # ============ Pool: quat products + A(qm) ==========================
        # off-diagonal col targets / G sources (col layout 4j+i, baseline)
        def build_A_offdiag(eng, E_t, G):
            eng.tensor_sub(E_t[:, 4:5], G[1][:, 2:3], G[0][:, 3:4])   # A01
            eng.tensor_add(E_t[:, 8:9], G[1][:, 3:4], G[0][:, 2:3])   # A02
            eng.tensor_add(E_t[:, 1:2], G[1][:, 2:3], G[0][:, 3:4])   # A10
            eng.tensor_sub(E_t[:, 9:10], G[2][:, 3:4], G[0][:, 1:2])  # A12
            eng.tensor_sub(E_t[:, 2:3], G[1][:, 3:4], G[0][:, 2:3])   # A20
            eng.tensor_add(E_t[:, 6:7], G[2][:, 3:4], G[0][:, 1:2])   # A21

        def build_A_diag(eng, E_t, G):
            # Aii = G0[0] + Gi[i] - 2  for i=1..3 at cols {0,5,10}
            for col, Gi, gi in ((0, G[1], 1), (5, G[2], 2), (10, G[3], 3)):
                tdg = st(B, 1)
                eng.tensor_add(tdg[:], G[0][:, 0:1], Gi[:, gi:gi + 1])
                eng.tensor_scalar(E_t[:, col:col + 1], tdg[:], -2.0,
                                  None, OP.add)

        def build_G(eng, q_t, inv2_t):
            # Gk = q * (q_k * 2/n2) via dual-scalar tensor_scalar
            G = []
            for k in range(4):
                Gk = st(B, 4)
                eng.tensor_scalar(Gk[:], q_t[:], q_t[:, k:k + 1], inv2_t,
                                  OP.mult, OP.mult)
                G.append(Gk)
            return G

        E = st(B, 16, tag="E")
        nc.gpsimd.memset(E[:], 0.0)
        acc32 = st(32, 1, tag="acc32")
        nc.gpsimd.memset(acc32[:], 0.0)
        SP = st(B, 16, tag="SPr")
        nc.gpsimd.tensor_mul(SP[:], ablk, rep16)
        twxyz = st(B, 4, tag="twxyz")
        nc.gpsimd.tensor_add(twxyz[:], SP[:, 0:4], SP[:, 4:8])
        nc.gpsimd.tensor_add(twxyz[:], twxyz[:], SP[:, 8:12])
        nc.gpsimd.tensor_add(twxyz[:], twxyz[:], SP[:, 12:16])
        SQ = st(B, 16, tag="SPq")
        nc.gpsimd.tensor_mul(SQ[:], bblk, rep16)
        qm = st(B, 4, tag="qm")
        nc.gpsimd.tensor_add(qm[:], SQ[:, 0:4], SQ[:, 4:8])
        nc.gpsimd.tensor_add(qm[:], qm[:], SQ[:, 8:12])
        nc.gpsimd.tensor_add(qm[:], qm[:], SQ[:, 12:16])
        # ================== flow + wedged pose work on DVE =================
        rsall = st(T_PAD, N_Q, tag="rsall")
        dsq = pcpool.tile([128, PC_COLS], BF16, tag="dsq")
        col_chunks = [(0, 512), (512, 1024), (1024, 1536), (1536, PC_COLS)]

        def flow_subs(b, d_t):
            pgb = pg_ts[b]
            for h in range(2):
                c0 = h * 2048
                nc.vector.tensor_sub(d_t[:, c0:c0 + 2048],
                                     pgb[:, 2 * c0:2 * c0 + 2048],
                                     pgb[:, 2 * c0 + 2048:2 * c0 + 4096])

        def flow_muls(b, d_t):
            for h in range(2):
                c0 = h * 2048
                nc.vector.tensor_mul(d_t[:, c0:c0 + 2048],
                                     d_t[:, c0:c0 + 2048], v_ts[b][:])
            for h in range(2):
                iq = 2 * b + h
                c0 = h * 2048
                scr = dpool.tile([T_PAD, 2048], BF16, tag="scr")
                nc.scalar.activation(scr[:], d_t[:, c0:c0 + 2048], AF.Abs,
                                     scale=wr[:],
                                     accum_out=rsall[:, iq:iq + 1])

        d0 = dpool.tile([T_PAD, 4096], BF16, tag="d")
        flow_subs(0, d0)
        # wedge: norms/recips (feeds Pool's Gq and the A(re) wedge below)
        sef = st(B, 8)
        nc.vector.tensor_mul(sef[:], retr8, retr8)
        nn = st(B, 2)   # [n2f, n2e]
        nc.vector.tensor_reduce(nn[:], sef[:].rearrange("b (g k) -> b g k", k=4),
                                AX.X, OP.add)
        rr = st(B, 2)   # [1/n2f, 1/n2e]
        nc.vector.reciprocal(rr[:], nn[:])
        rnq2 = st(B, 1)  # 2/(n2e*n2f)
        nc.vector.tensor_mul(rnq2[:], rr[:, 0:1], rr[:, 1:2])
        nc.vector.tensor_scalar(rnq2[:], rnq2[:], 2.0, None, OP.mult)
        rne2 = st(B, 1)  # 2/n2e
        nc.vector.tensor_scalar(rne2[:], rr[:, 1:2], 2.0, None, OP.mult)
        flow_muls(0, d0)

        # Pool continues: A(qm) into E (waits on rnq2 via semaphore)
        Gq = build_G(nc.gpsimd, qm[:], rnq2[:])
        build_A_offdiag(nc.gpsimd, E[:], Gq)
        build_A_diag(nc.gpsimd, E[:], Gq)
        sqt = st(B, 4)
        nc.gpsimd.tensor_mul(sqt[:], twxyz[:], twxyz[:])
        vn2 = st(B, 1)
        nc.gpsimd.tensor_add(vn2[:], sqt[:, 1:2], sqt[:, 2:3])
        nc.gpsimd.tensor_add(vn2[:], vn2[:], sqt[:, 3:4])
        aw2 = sqt[:, 0:1]

        d1 = dpool.tile([T_PAD, 4096], BF16, tag="d")
        flow_subs(1, d1)
        u = st(B, 3)
        nc.vector.tensor_sub(u[:], tt_s, te_s)
        z128 = st(128, 128, tag="z128")
        nc.vector.memset(z128[:], 0.0)
        l2d = dram.tile([128, 128], F32)
        nc.sync.dma_start(l2d[:], z128[:])
        flow_muls(1, d1)

        d2 = dpool.tile([T_PAD, 4096], BF16, tag="d")
        flow_subs(2, d2)
        # wedge: A(re) -> AE (feeds Mt only)
        AE = st(B, 16, tag="AE")
        nc.vector.memset(AE[:], 0.0)
        Gr = build_G(nc.vector, re_s, rne2[:])
        build_A_offdiag(nc.vector, AE[:], Gr)
        build_A_diag(nc.vector, AE[:], Gr)
        flow_muls(2, d2)

        # wedge: Mt = u + Ae^T u, then E8 replicate + single scatter bounce
        aev = AE[:].rearrange("b (j i) -> b j i", i=4)
        nc.vector.scalar_tensor_tensor(E[:, 12:15], aev[:, 0:3, 0],
                                       u[:, 0:1], u[:], OP.mult, OP.add)
        nc.vector.scalar_tensor_tensor(E[:, 12:15], aev[:, 0:3, 1],
                                       u[:, 1:2], E[:, 12:15], OP.mult, OP.add)
        nc.vector.scalar_tensor_tensor(E[:, 12:15], aev[:, 0:3, 2],
                                       u[:, 2:3], E[:, 12:15], OP.mult, OP.add)
        E8 = st(B, 128, tag="E8")
        nc.vector.tensor_scalar(E8[:, 0:16], E[:], 1.0, None, OP.mult)
        nc.vector.tensor_scalar(E8[:, 16:32], E8[:, 0:16], 1.0, None, OP.mult)
        nc.vector.tensor_scalar(E8[:, 32:64], E8[:, 0:32], 1.0, None, OP.mult)
        nc.vector.tensor_scalar(E8[:, 64:128], E8[:, 0:64], 1.0, None,
                                OP.mult)
        e8v = E8[:].rearrange("b (g j i) -> b g j i", j=4, i=4)
        l2d_ap = l2d[:]
        dstg = bass.AP(l2d_ap.tensor, 0,
                       [[2064, PC_GROUPS], [516, 4], [128, 4], [1, 4]])
        nc.sync.dma_start(dstg, e8v)
        lhsT2 = st(128, 128, tag="lhsT2")
        nc.sync.dma_start(lhsT2[:], l2d[:])

        d3 = dpool.tile([T_PAD, 4096], BF16, tag="d")
        flow_subs(3, d3)
        flow_muls(3, d3)

        # ---- DVE-late: loss_transl + rot-angle prep ----------------------
        dlt = st(B, 3)
        nc.vector.tensor_sub(dlt[:], te_s, tt_s)
        sc1 = st(B, 3)
        nc.vector.tensor_mul(sc1[:], dlt[:], dlt[:])
        s12 = st(B, 2)
        nc.vector.tensor_reduce(s12[:, 0:1], sc1[:], AX.X, OP.add)
        adl = st(B, 3)
        nc.vector.scalar_tensor_tensor(adl[:], dlt[:], -1.0, dlt[:],
                                       OP.mult, OP.max)
        rdl = st(B, 3)
        nc.vector.tensor_scalar(rdl[:], adl[:], 1.0, 0.0, OP.subtract, OP.max)
        sc2 = st(B, 3)
        nc.vector.tensor_mul(sc2[:], rdl[:], rdl[:])
        nc.vector.tensor_reduce(s12[:, 1:2], sc2[:], AX.X, OP.add)
        ltd = st(B, 1, tag="ltd")   # 2*smooth_l1 row sum per batch
        nc.vector.tensor_sub(ltd[:], s12[:, 0:1], s12[:, 1:2])

        mn2 = st(B, 1)
        nc.vector.tensor_tensor(mn2[:], vn2[:], aw2, OP.min)
        mx2 = st(B, 1)
        nc.vector.tensor_max(mx2[:], vn2[:], aw2)
        rmx = st(B, 1)
        nc.vector.reciprocal(rmx[:], mx2[:])
        rat2 = st(B, 1)
        nc.vector.tensor_mul(rat2[:], mn2[:], rmx[:])
        mflip = st(B, 1, dt=mybir.dt.int32)
        nc.vector.tensor_tensor(mflip[:], vn2[:], aw2, OP.is_gt)

        # ============ point-cloud chain (PE + ScalarE, post-flow) ==========
        for c0, c1 in col_chunks:
            dps = psum_d.tile([128, 512], F32, tag="dps")
            nc.tensor.matmul(dps[:, :c1 - c0], lhsT2[:], pcp[:, c0:c1],
                             start=True, stop=True)
            nc.scalar.activation(dsq[:, c0:c1], dps[:, :c1 - c0], AF.Square)
        for c0, c1 in col_chunks:
            e2 = psum_e.tile([32, 512], F32, tag="e2")
            nc.tensor.matmul(e2[:, :c1 - c0], lhsT3[:], dsq[:, c0:c1],
                             start=True, stop=True)
            errt = pwork.tile([32, 512], F32, tag="errt")
            ers = pwork.tile([32, 1], F32, tag="ers")
            nc.scalar.activation(errt[:, :c1 - c0], e2[:, :c1 - c0], AF.Sqrt,
                                 accum_out=ers[:])
            nc.gpsimd.tensor_add(acc32[:], acc32[:], ers[:])

        # ---- deferred pose-angle tail (Sqrt in-set, one Arctan load) ------
        rat = st(B, 1)
        nc.scalar.activation(rat[:], rat2[:], AF.Sqrt)
        ang = st(B, 1)
        nc.scalar.activation(ang[:], rat[:], AF.Arctan)
        alt = st(B, 1)
        nc.gpsimd.tensor_scalar(alt[:], ang[:], -1.0, HALF_PI, OP.mult, OP.add)
        rot = st(B, 1, tag="rot")   # atan2 per batch
        nc.vector.select(rot[:], mflip[:], alt[:], ang[:])

        # ================== final reductions ===============================
        ones128 = st(T_PAD, 1, tag="ones128")
        nc.gpsimd.memset(ones128[:], 1.0)
        ones4 = st(B, 1, tag="ones4")
        nc.gpsimd.memset(ones4[:], 1.0)
        ones32 = st(32, 1, tag="ones32")
        nc.gpsimd.memset(ones32[:], 1.0)

        # flow: [128,8] --rowsum--> [128,1] --matmul--> [1,1]
        rsum = st(T_PAD, 1, tag="rsum")
        nc.vector.tensor_reduce(rsum[:], rsall[:], AX.X, OP.add)

        ps = psum_s.tile([1, 5], F32, tag="ps")
        nc.tensor.matmul(ps[:, 0:1], rsum[:], ones128[:], start=True, stop=True)
        nc.tensor.matmul(ps[:, 1:2], acc32[:], ones32[:], start=True, stop=True)
        nc.tensor.matmul(ps[:, 2:3], ltd[:], ones4[:], start=True, stop=True)
        nc.tensor.matmul(ps[:, 3:4], rot[:], ones4[:], start=True, stop=True)

        out5 = st(1, 5, tag="out5")
        # loss_transl = 0.5*sum/4 ; loss_rot = 2*sum/4 ; pc = sum/(B*N) ; flow
        nc.scalar.mul(out5[:, 1:2], ps[:, 2:3], 0.125)
        nc.scalar.mul(out5[:, 2:3], ps[:, 3:4], 0.5)
        nc.scalar.mul(out5[:, 3:4], ps[:, 1:2], 1.0 / (B * N_PTS))
        nc.scalar.copy(out5[:, 4:5], ps[:, 0:1])
        t1 = st(1, 1)
        t2 = st(1, 1)
        nc.gpsimd.tensor_add(t1[:], out5[:, 1:2], out5[:, 2:3])
        nc.gpsimd.tensor_add(t2[:], out5[:, 3:4], out5[:, 4:5])
        nc.gpsimd.tensor_scalar(t1[:], t1[:], 0.5 / N_CORES, None, OP.mult)
        nc.vector.scalar_tensor_tensor(out5[:, 0:1], t2[:], 0.5, t1[:],
                                       OP.mult, OP.add)
        nc.sync.dma_start(out[:], out5[:])


_CACHE = {}
last_results = None


def _get_nc():
    if "nc" not in _CACHE:
        _CACHE["nc"] = build_nc()
    return _CACHE["nc"]


def _signed_blocks(r):
    """r: [B,4] -> [B,32] = sign-permuted copies for the two quat products.

    A-block (rot_err x conj(target_rot), component-ordered):
      A0=(r0,-r1,-r2,-r3)  A1=(r1,r0,r3,-r2)  A2=(r2,-r3,r0,r1)  A3=(r3,r2,-r1,r0)
    B-block (conj(rot_err) x target_rot):
      B0=(r0,r1,r2,r3)  B1=(r1,-r0,r3,-r2)  B2=(r2,-r3,-r0,r1)  B3=(r3,r2,-r1,-r0)
    """
    r0, r1, r2, r3 = r[:, 0:1], r[:, 1:2], r[:, 2:3], r[:, 3:4]
    a = np.concatenate([r0, -r1, -r2, -r3,
                        r1, r0, r3, -r2,
                        r2, -r3, r0, r1,
                        r3, r2, -r1, r0], axis=1)
    b = np.concatenate([r0, r1, r2, r3,
                        r1, -r0, r3, -r2,
                        r2, -r3, -r0, r1,
                        r3, r2, -r1, -r0], axis=1)
    return np.concatenate([a, b], axis=1)


def make_in_maps(point_clouds, target_transl, target_rot, transl_err, rot_err,
                 calib_flow_pred, calib_flow_gt, flow_valid):
    import ml_dtypes
    point_clouds = np.asarray(point_clouds, np.float32)
    calib_flow_pred = np.asarray(calib_flow_pred, np.float32)
    calib_flow_gt = np.asarray(calib_flow_gt, np.float32)
    flow_valid = np.asarray(flow_valid, np.float32)
    tt = np.ascontiguousarray(np.asarray(target_transl, np.float32))
    tr = np.ascontiguousarray(np.asarray(target_rot, np.float32))
    te = np.ascontiguousarray(np.asarray(transl_err, np.float32))
    re = np.ascontiguousarray(np.asarray(rot_err, np.float32))

    w_full = (GAMMA ** (N_ITERS - 1 - np.arange(N_ITERS, dtype=np.float64)))
    w_full = (w_full / FLOW_MEAN_DEN).astype(np.float32)

    re_rep16 = np.repeat(re, 4, axis=1)          # [B,16]: re[i] at 4i..4i+3
    retr8 = np.concatenate([tr, re], axis=1)     # [B,8]
    smalls = np.concatenate([tt, tr, te, re, _signed_blocks(tr),
                             re_rep16, retr8], axis=1).astype(np.float32)

    # [B,1000,2,32,64] -> per-core rows (b,t), cols pred(c,hw) | gt(c,hw)
    pred8 = calib_flow_pred.reshape(B, N_ITERS, 4096).astype(
        ml_dtypes.float8_e4m3)
    gt8 = calib_flow_gt.reshape(B, N_ITERS, 4096).astype(
        ml_dtypes.float8_e4m3)
    valid16 = flow_valid.reshape(B, N_ITERS, 2048).astype(ml_dtypes.bfloat16)

    in_maps = []
    for c in range(N_CORES):
        t0, t1 = c * T_PER_CORE, (c + 1) * T_PER_CORE
        n0, n1 = c * PTS_PER_CORE, (c + 1) * PTS_PER_CORE
        p4 = pred8[:, t0:t1].reshape(B, T_PER_CORE, 2, 2048)
        g4 = gt8[:, t0:t1].reshape(B, T_PER_CORE, 2, 2048)
        pg_s = np.stack([p4[:, :, 0], g4[:, :, 0],
                         p4[:, :, 1], g4[:, :, 1]], axis=2)
        # pad each b-block from 125 to 128 t-rows; the zero rows (and the
        # zero wrow entries) contribute nothing to any reduction
        pg_pad = np.zeros((B, T_PAD, 8192), pg_s.dtype)
        pg_pad[:, :T_PER_CORE] = pg_s.reshape(B, T_PER_CORE, 8192)
        v_pad = np.zeros((B, T_PAD, 2048), valid16.dtype)
        v_pad[:, :T_PER_CORE] = valid16[:, t0:t1]
        w_pad = np.zeros((T_PAD, 1), np.float32)
        w_pad[:T_PER_CORE, 0] = w_full[t0:t1]
        in_maps.append({
            "pg": np.ascontiguousarray(pg_pad).reshape(ROWS, 8192),
            "valid": np.ascontiguousarray(v_pad).reshape(ROWS, 2048),
            "wrow": w_pad,
            "pc": _pack_pc(point_clouds[:, :, n0:n1]),
            "smalls": smalls,
        })
    return in_maps


def _pack_pc(pc_shard):
    """[B,4,12500] -> [128,1568]: row 16g+4b+j = pc[b,j,1568g:1568(g+1)],
    zero-padded to 12544 points (zero points contribute zero error)."""
    pad = np.zeros((B, 4, PAD_N), np.float32)
    pad[:, :, :PTS_PER_CORE] = pc_shard
    v = pad.reshape(B, 4, PC_GROUPS, PC_COLS)
    return np.ascontiguousarray(
        v.transpose(2, 0, 1, 3).reshape(16 * PC_GROUPS, PC_COLS))


def combine_outputs(core_outs):
    """core_outs: [N_CORES, 5] array of per-core partials."""
    core_outs = np.asarray(core_outs, np.float32)
    total = np.float32(core_outs[:, 0].sum())
    lt = np.float32(core_outs[0, 1])
    lr = np.float32(core_outs[0, 2])
    pcb = np.float32(core_outs[:, 3].sum())
    fl = np.float32(core_outs[:, 4].sum())
    return (total, lt, lr, pcb, fl)


def _install_ntff_hook_shim():
    """bass_utils expects antenv.axon_hooks when trace=True under axon;
    this image's antenv lacks it. Provide it and register the ctypes hook."""
    import sys
    import types
    if "antenv.axon_hooks" in sys.modules:
        return
    mod = types.ModuleType("antenv.axon_hooks")
    state = {"hook": None}
    mod.set_axon_ntff_profile_hook = lambda h: state.__setitem__("hook", h)
    mod.get_axon_ntff_profile_hook = lambda: state["hook"]
    sys.modules["antenv.axon_hooks"] = mod
    try:
        import antenv
        antenv.axon_hooks = mod
    except ImportError:
        pass
    try:
        from trn_agent_boot.trn_boot import _ntff_profile_via_ctypes
        mod.set_axon_ntff_profile_hook(
            _ntff_profile_via_ctypes("/opt/axon/libaxon_pjrt.so"))
    except Exception:
        pass


def kernel(point_clouds, target_transl, target_rot, transl_err, rot_err,
           calib_flow_pred, calib_flow_gt, flow_valid):
    global last_results
    from concourse.bass_utils import run_bass_kernel_spmd

    nc = _get_nc()
    in_maps = make_in_maps(point_clouds, target_transl, target_rot,
                           transl_err, rot_err, calib_flow_pred,
                           calib_flow_gt, flow_valid)
    trace = bool(int(os.environ.get("KERNEL_TRACE", "0")))
    kwargs = {}
    if trace:
        _install_ntff_hook_shim()
        kwargs = {"trace": True, "trace_cores": list(range(N_CORES))}
    res = run_bass_kernel_spmd(nc, in_maps, core_ids=list(range(N_CORES)),
                               **kwargs)
    last_results = res
    core_outs = np.stack([res.results[c]["out"][0] for c in range(N_CORES)])
    return combine_outputs(core_outs)
